# revision 53
# baseline (speedup 1.0000x reference)
"""Chamfer loss kernel for Trainium2, batch-parallel over 8 NeuronCores.

Per core (one batch element b):
  gts = src_points[b] @ R^T + t          (host, fp64)
  P[i,j] = |gts_i|^2 + |recon_j|^2 - 2 gts_i . recon_j
  loss_b = sum_j min_i P + sum_i min_j P
Host sums the 8 partial losses.

Structure (v4):
- The host assembles the two K=11 bf16 matmul operands directly:
  rows 0-8 are the -2*g.p cross terms in a bf16 hi/lo pair decomposition
  (hi*hi + hi*lo + lo*hi, ~2^-18 relative accuracy), row 9 carries the
  yy_j norm (ones x yy), row 10 carries the xx_i norm (xx x ones). The
  norms are single bf16; their error is row/column-structured and cancels
  to ~1e-4 in the summed loss. The device sees ready operands, so the
  whole on-device prep phase is two ~90KB DMA loads.
- ONE K=11 bf16 matmul per 512-col chunk produces the COMPLETE distance
  tile in PSUM.
- PSUM exit: nearly all half-blocks leave PSUM through the ACT engine
  (copy -> bf16 SBUF) whose 1 elem/cyc/lane stream is the saturated
  bottleneck; per-row mins ride along on DVE tensor_scalar ops with a
  min accum_out (4x mode on the staged bf16, 0.25 cyc/elem), and the
  running per-column min is a 2x bf16 tensor_tensor per block, lagged
  one block so it never waits on the current block's exits. The last
  block's h0 exits through DVE tensor_scalar (stage + row-min in one
  1x op) so the final dependency chain is short.
- Per-column mins are finished with PE transposes + free-axis folds,
  pipelined against the last block's quarter-merges; the final
  partition sums ([128,2]) are added on the host.
"""

import os

# the axon client here has no NTFF profile hook; a stray BASS_TRACE=1 in the
# environment would crash run_bass_kernel_spmd on a missing import
os.environ["BASS_NEVER_TRACE"] = "1"

import ml_dtypes
import numpy as np

import concourse.bacc as bacc
import concourse.bass as bass
import concourse.mybir as mybir
import concourse.tile as tile
from concourse.bass_utils import run_bass_kernel_spmd

F32 = mybir.dt.float32
BF16 = mybir.dt.bfloat16
ALU = mybir.AluOpType
AX = mybir.AxisListType

N_CORES = 8
NPTS = 4096          # points per set (both gts and recon)
NBLK = NPTS // 128   # 32 row blocks
HALF = 2048          # P tile free width (4 PSUM banks)

# (block, half) units force-staged by DVE tensor_scalar (stage + row-min
# fused, 1x from PSUM). Empty measures best: the PSUM exits are emitted
# engine-unassigned and the tile scheduler splits them between ACT and
# DVE better than any hand-picked fusion set.
DVE_EXIT_HALVES = frozenset()

_CACHE = {}
LAST_RESULTS = None


def _build_kernel():
    nc = bacc.Bacc("TRN2", target_bir_lowering=False, debug=False)

    lhsd = nc.declare_dram_parameter("lhsd", [11, NPTS], BF16, isOutput=False)
    rhsd = nc.declare_dram_parameter("rhsd", [11, NPTS], BF16, isOutput=False)
    ident = nc.declare_dram_parameter("ident", [128, 128], BF16, isOutput=False)
    partial = nc.declare_dram_parameter("partial", [128, 2], F32, isOutput=True)

    with tile.TileContext(nc) as tc:
        with tc.tile_pool(name="sb", bufs=1) as sb:
            # ---- phase 0: load operands (two parallel DMA queues) -------
            # rhs rides the SP queue, lhs the ACT queue (idle this early);
            # the first distance matmuls need lhs cols 0:128 + rhs 0:2048
            lhs = sb.tile([11, NPTS], BF16)
            rhs = sb.tile([11, NPTS], BF16)
            ident_sb = sb.tile([128, 128], BF16)
            nc.sync.dma_start(out=ident_sb[:, :], in_=ident[:, :])
            nc.sync.dma_start(out=rhs[:, 0:2048], in_=rhsd[:, 0:2048])
            nc.scalar.dma_start(out=lhs[:, 0:2048], in_=lhsd[:, 0:2048])
            nc.sync.dma_start(out=rhs[:, 2048:4096], in_=rhsd[:, 2048:4096])
            nc.scalar.dma_start(out=lhs[:, 2048:4096], in_=lhsd[:, 2048:4096])

            # running reduction state (mrun needs no seed: block 0 copies
            # into it; rminA/B columns are fully written per block)
            mrun = sb.tile([128, NPTS], BF16)    # running col-min
            rminA = sb.tile([128, NBLK], F32)    # per-block h0 row mins
            rminB = sb.tile([128, NBLK], F32)    # per-block h1 row mins

            # PE warm-up on the identity while operands load, so the main
            # matmul stream starts at full PE clock
            with tc.tile_pool(name="warm_ps", bufs=1, space="PSUM") as wpp:
                warm_ps = wpp.tile([128, 128], F32)
                for _ in range(24):
                    nc.tensor.matmul(warm_ps[:, :], lhsT=ident_sb[:, :],
                                     rhs=ident_sb[:, :], start=True,
                                     stop=True)

            # ---- phase 1: distance tiles + fused min reductions ---------
            junk = sb.tile([128, NPTS], BF16)   # throwaway TS main output

            sg_last = sb.tile([128, NPTS], BF16)   # block 31 outlives pools

            with tc.tile_pool(name="stage_sb", bufs=8) as stg, \
                 tc.tile_pool(name="main_ps", bufs=2, space="PSUM") as mps:
                prev_sg = None
                for b in range(NBLK):
                    if b == NBLK - 1:
                        sg = sg_last
                        # block 30 is fully staged by now; merge it before
                        # block 31's DVE exit so the tail chain is shorter
                        nc.any.tensor_tensor(mrun[:, :], prev_sg[:, :],
                                             mrun[:, :], ALU.min)
                    else:
                        sg = stg.tile([128, NPTS], BF16, tag="SG")
                    lw = lhs[:, b * 128:(b + 1) * 128]
                    for h in range(2):
                        pt = mps.tile([128, HALF], F32, tag="P")
                        for s in range(HALF // 512):
                            j0 = h * HALF + s * 512
                            nc.tensor.matmul(
                                pt[:, s * 512:(s + 1) * 512], lhsT=lw,
                                rhs=rhs[:, j0:j0 + 512],
                                start=True, stop=True)
                        hs = slice(h * HALF, (h + 1) * HALF)
                        rm = rminA if h == 0 else rminB
                        # stage to bf16 AND accumulate the per-row min in
                        # ONE engine-unassigned op: on whichever engine the
                        # scheduler picks, the row-min rides the mandatory
                        # PSUM exit for free
                        nc.any.tensor_scalar(
                            sg[:, hs], pt[:, :], 0.0, None, ALU.add,
                            ALU.min, accum_out=rm[:, b:b + 1])
                        # col-min merge, lagged one block: while ACT exits
                        # block b, DVE merges block b-1 (no dependency on
                        # the current block, so DVE never waits on ACT)
                        if h == 0 and prev_sg is not None and b < NBLK - 1:
                            if b == 1:
                                nc.vector.tensor_copy(mrun[:, :],
                                                      prev_sg[:, :])
                            else:
                                nc.any.tensor_tensor(mrun[:, :],
                                                     prev_sg[:, :],
                                                     mrun[:, :], ALU.min)
                    prev_sg = sg

            # ---- phase 2: finishers -------------------------------------
            # per-partition sums go to the host, which adds the 256 floats
            psums = sb.tile([128, 2], F32)
            cmin = sb.tile([128, NPTS // 128], F32)

            with tc.tile_pool(name="fin_ps", bufs=4, space="PSUM") as fps:
                # block 31 merges in column quarters; each quarter's
                # transposes + fold start while the next quarter merges
                tps = []
                for g in range(4):
                    qs = slice(g * 1024, (g + 1) * 1024)
                    nc.vector.tensor_tensor(mrun[:, qs], sg_last[:, qs],
                                            mrun[:, qs], ALU.min)
                    tp = fps.tile([128, 1024], BF16, tag="T")
                    for c in range(8):
                        j0 = (g * 8 + c) * 128
                        nc.tensor.transpose(tp[:, c * 128:(c + 1) * 128],
                                            mrun[:, j0:j0 + 128],
                                            ident_sb[:, :])
                    tps.append(tp)
                # row-side combine slots into the transpose wait
                nc.vector.tensor_tensor(rminA[:, :], rminA[:, :],
                                        rminB[:, :], ALU.min)
                nc.vector.tensor_reduce(psums[:, 0:1], rminA[:, :],
                                        axis=AX.X, op=ALU.add)
                nc.sync.dma_start(out=partial[:, 0:1], in_=psums[:, 0:1])
                for g in range(4):
                    nc.vector.tensor_reduce(
                        cmin[:, 8 * g:8 * g + 8],
                        tps[g].rearrange("p (g w) -> p g w", w=128),
                        axis=AX.X, op=ALU.min)
                nc.vector.tensor_reduce(psums[:, 1:2], cmin[:, :], axis=AX.X,
                                        op=ALU.add)

            nc.sync.dma_start(out=partial[:, 1:2], in_=psums[:, 1:2])

    nc.compile()
    return nc


def _bf16(x):
    return x.astype(ml_dtypes.bfloat16)


def _prep_core_inputs(recon_b, src_b, transform_b):
    # transform on host at fp64: gts = src @ R^T + t
    R = transform_b[:3, :3].astype(np.float64)
    t = transform_b[:3, 3].astype(np.float64)
    gts = src_b.astype(np.float64) @ R.T + t            # [N, 3]
    rec = recon_b.astype(np.float64)                    # [M, 3]

    xx = np.sum(gts * gts, axis=1)                      # [N]
    yy = np.sum(rec * rec, axis=1)                      # [M]

    # bf16 hi/lo pair decomposition of the cross-term factors
    g2 = (-2.0 * gts).astype(np.float32)                # [N, 3]
    g_hi = _bf16(g2)
    g_lo = _bf16(g2 - g_hi.astype(np.float32))
    p32 = rec.astype(np.float32)
    p_hi = _bf16(p32)
    p_lo = _bf16(p32 - p_hi.astype(np.float32))

    lhs = np.empty((11, NPTS), ml_dtypes.bfloat16)
    rhs = np.empty((11, NPTS), ml_dtypes.bfloat16)
    lhs[0:3] = g_hi.T
    lhs[3:6] = g_hi.T
    lhs[6:9] = g_lo.T
    lhs[9] = ml_dtypes.bfloat16(1.0)
    lhs[10] = _bf16(xx.astype(np.float32))
    rhs[0:3] = p_hi.T
    rhs[3:6] = p_lo.T
    rhs[6:9] = p_hi.T
    rhs[9] = _bf16(yy.astype(np.float32))
    rhs[10] = ml_dtypes.bfloat16(1.0)

    return {
        "lhsd": np.ascontiguousarray(lhs),
        "rhsd": np.ascontiguousarray(rhs),
        "ident": np.eye(128).astype(ml_dtypes.bfloat16),
    }


def kernel(recon, src_points, transform):
    global LAST_RESULTS
    recon = np.asarray(recon, np.float32)
    src_points = np.asarray(src_points, np.float32)
    transform = np.asarray(transform, np.float32)
    B = recon.shape[0]
    assert B == N_CORES

    if "nc" not in _CACHE:
        _CACHE["nc"] = _build_kernel()
    nc = _CACHE["nc"]

    in_maps = [
        _prep_core_inputs(recon[b], src_points[b], transform[b])
        for b in range(B)
    ]
    res = run_bass_kernel_spmd(nc, in_maps, list(range(N_CORES)))
    LAST_RESULTS = res
    total = np.float64(0.0)
    for r in res.results:
        total += np.float64(np.sum(r["partial"].astype(np.float64)))
    return np.float32(total)


# revision 54
# speedup vs baseline: 1.6762x; 1.6762x over previous
"""Chamfer loss kernel for Trainium2, batch-parallel over 8 NeuronCores.

Per core (one batch element b):
  gts = src_points[b] @ R^T + t          (host, fp64)
  P[i,j] = |gts_i|^2 + |recon_j|^2 - 2 gts_i . recon_j
  loss_b = sum_j min_i P + sum_i min_j P
Host sums the 8 partial losses.

Structure (v4):
- The host assembles the two K=11 bf16 matmul operands directly:
  rows 0-8 are the -2*g.p cross terms in a bf16 hi/lo pair decomposition
  (hi*hi + hi*lo + lo*hi, ~2^-18 relative accuracy), row 9 carries the
  yy_j norm (ones x yy), row 10 carries the xx_i norm (xx x ones). The
  norms are single bf16; their error is row/column-structured and cancels
  to ~1e-4 in the summed loss. The device sees ready operands, so the
  whole on-device prep phase is two ~90KB DMA loads.
- ONE K=11 bf16 matmul per 512-col chunk produces the COMPLETE distance
  tile in PSUM.
- PSUM exit: nearly all half-blocks leave PSUM through the ACT engine
  (copy -> bf16 SBUF) whose 1 elem/cyc/lane stream is the saturated
  bottleneck; per-row mins ride along on DVE tensor_scalar ops with a
  min accum_out (4x mode on the staged bf16, 0.25 cyc/elem), and the
  running per-column min is a 2x bf16 tensor_tensor per block, lagged
  one block so it never waits on the current block's exits. The last
  block's h0 exits through DVE tensor_scalar (stage + row-min in one
  1x op) so the final dependency chain is short.
- Per-column mins are finished with PE transposes + free-axis folds,
  pipelined against the last block's quarter-merges; the final
  partition sums ([128,2]) are added on the host.
"""

import os

# the axon client here has no NTFF profile hook; a stray BASS_TRACE=1 in the
# environment would crash run_bass_kernel_spmd on a missing import
os.environ["BASS_NEVER_TRACE"] = "1"

import ml_dtypes
import numpy as np

import concourse.bacc as bacc
import concourse.bass as bass
import concourse.mybir as mybir
import concourse.tile as tile
from concourse.bass_utils import run_bass_kernel_spmd

F32 = mybir.dt.float32
BF16 = mybir.dt.bfloat16
ALU = mybir.AluOpType
AX = mybir.AxisListType

N_CORES = 8
NPTS = 4096          # points per set (both gts and recon)
NBLK = NPTS // 128   # 32 row blocks
HALF = 2048          # P tile free width (4 PSUM banks)

# (block, half) units force-staged by DVE tensor_scalar (stage + row-min
# fused, 1x from PSUM). Empty measures best: the PSUM exits are emitted
# engine-unassigned and the tile scheduler splits them between ACT and
# DVE better than any hand-picked fusion set.
DVE_EXIT_HALVES = frozenset()

_CACHE = {}
LAST_RESULTS = None


def _build_kernel():
    nc = bacc.Bacc("TRN2", target_bir_lowering=False, debug=False)

    lhsd = nc.declare_dram_parameter("lhsd", [11, NPTS], BF16, isOutput=False)
    rhsd = nc.declare_dram_parameter("rhsd", [11, NPTS], BF16, isOutput=False)
    ident = nc.declare_dram_parameter("ident", [128, 128], BF16, isOutput=False)
    partial = nc.declare_dram_parameter("partial", [128, 2], F32, isOutput=True)

    with tile.TileContext(nc) as tc:
        with tc.tile_pool(name="sb", bufs=1) as sb:
            # ---- phase 0: load operands (two parallel DMA queues) -------
            # rhs rides the SP queue, lhs the ACT queue (idle this early);
            # the first distance matmuls need lhs cols 0:128 + rhs 0:2048
            lhs = sb.tile([11, NPTS], BF16)
            rhs = sb.tile([11, NPTS], BF16)
            ident_sb = sb.tile([128, 128], BF16)
            nc.sync.dma_start(out=ident_sb[:, :], in_=ident[:, :])
            nc.sync.dma_start(out=rhs[:, 0:2048], in_=rhsd[:, 0:2048])
            nc.scalar.dma_start(out=lhs[:, 0:2048], in_=lhsd[:, 0:2048])
            nc.sync.dma_start(out=rhs[:, 2048:4096], in_=rhsd[:, 2048:4096])
            nc.scalar.dma_start(out=lhs[:, 2048:4096], in_=lhsd[:, 2048:4096])

            # running reduction state (mrun needs no seed: block 0 copies
            # into it; rminA/B columns are fully written per block)
            mrun = sb.tile([128, NPTS], BF16)    # running col-min
            rminA = sb.tile([128, NBLK], F32)    # per-block h0 row mins
            rminB = sb.tile([128, NBLK], F32)    # per-block h1 row mins

            # PE warm-up on the identity while operands load, so the main
            # matmul stream starts at full PE clock
            with tc.tile_pool(name="warm_ps", bufs=1, space="PSUM") as wpp:
                warm_ps = wpp.tile([128, 128], F32)
                for _ in range(24):
                    nc.tensor.matmul(warm_ps[:, :], lhsT=ident_sb[:, :],
                                     rhs=ident_sb[:, :], start=True,
                                     stop=True)

            # ---- phase 1: distance tiles + fused min reductions ---------
            junk = sb.tile([128, NPTS], BF16)   # throwaway TS main output

            sg_last = sb.tile([128, NPTS], BF16)   # block 31 outlives pools

            with tc.tile_pool(name="stage_sb", bufs=8) as stg, \
                 tc.tile_pool(name="main_ps", bufs=2, space="PSUM") as mps:
                prev_sg = None
                for b in range(NBLK):
                    if b == NBLK - 1:
                        sg = sg_last
                        # block 30 is fully staged by now; merge it before
                        # block 31's DVE exit so the tail chain is shorter
                        nc.any.tensor_tensor(mrun[:, :], prev_sg[:, :],
                                             mrun[:, :], ALU.min)
                    else:
                        sg = stg.tile([128, NPTS], BF16, tag="SG")
                    lw = lhs[:, b * 128:(b + 1) * 128]
                    for h in range(2):
                        pt = mps.tile([128, HALF], F32, tag="P")
                        for s in range(HALF // 512):
                            j0 = h * HALF + s * 512
                            nc.tensor.matmul(
                                pt[:, s * 512:(s + 1) * 512], lhsT=lw,
                                rhs=rhs[:, j0:j0 + 512],
                                start=True, stop=True)
                        hs = slice(h * HALF, (h + 1) * HALF)
                        rm = rminA if h == 0 else rminB
                        if (b, h) in DVE_EXIT_HALVES:
                            # stage to bf16 + per-row min in one DVE op
                            nc.vector.tensor_scalar(
                                sg[:, hs], pt[:, :], 0.0, None, ALU.add,
                                ALU.min, accum_out=rm[:, b:b + 1])
                        else:
                            nc.any.tensor_copy(sg[:, hs], pt[:, :])
                            # half-row min at 4x DVE rate, overlapping the
                            # other half's ACT exit; main output discarded
                            nc.any.tensor_scalar(
                                junk[:, hs], sg[:, hs], 0.0, None, ALU.add,
                                ALU.min, accum_out=rm[:, b:b + 1])
                        # col-min merge, lagged one block: while ACT exits
                        # block b, DVE merges block b-1 (no dependency on
                        # the current block, so DVE never waits on ACT)
                        if h == 0 and prev_sg is not None and b < NBLK - 1:
                            if b == 1:
                                nc.vector.tensor_copy(mrun[:, :],
                                                      prev_sg[:, :])
                            else:
                                nc.any.tensor_tensor(mrun[:, :],
                                                     prev_sg[:, :],
                                                     mrun[:, :], ALU.min)
                    prev_sg = sg

            # ---- phase 2: finishers -------------------------------------
            # per-partition sums go to the host, which adds the 256 floats
            psums = sb.tile([128, 2], F32)
            cmin = sb.tile([128, NPTS // 128], F32)

            with tc.tile_pool(name="fin_ps", bufs=4, space="PSUM") as fps:
                # block 31 merges in column quarters; each quarter's
                # transposes + fold start while the next quarter merges
                tps = []
                for g in range(4):
                    qs = slice(g * 1024, (g + 1) * 1024)
                    nc.vector.tensor_tensor(mrun[:, qs], sg_last[:, qs],
                                            mrun[:, qs], ALU.min)
                    tp = fps.tile([128, 1024], BF16, tag="T")
                    for c in range(8):
                        j0 = (g * 8 + c) * 128
                        nc.tensor.transpose(tp[:, c * 128:(c + 1) * 128],
                                            mrun[:, j0:j0 + 128],
                                            ident_sb[:, :])
                    tps.append(tp)
                # row-side combine slots into the transpose wait
                nc.vector.tensor_tensor(rminA[:, :], rminA[:, :],
                                        rminB[:, :], ALU.min)
                nc.vector.tensor_reduce(psums[:, 0:1], rminA[:, :],
                                        axis=AX.X, op=ALU.add)
                nc.sync.dma_start(out=partial[:, 0:1], in_=psums[:, 0:1])
                for g in range(4):
                    nc.vector.tensor_reduce(
                        cmin[:, 8 * g:8 * g + 8],
                        tps[g].rearrange("p (g w) -> p g w", w=128),
                        axis=AX.X, op=ALU.min)
                nc.vector.tensor_reduce(psums[:, 1:2], cmin[:, :], axis=AX.X,
                                        op=ALU.add)

            nc.sync.dma_start(out=partial[:, 1:2], in_=psums[:, 1:2])

    nc.compile()
    return nc


def _bf16(x):
    return x.astype(ml_dtypes.bfloat16)


def _prep_core_inputs(recon_b, src_b, transform_b):
    # transform on host at fp64: gts = src @ R^T + t
    R = transform_b[:3, :3].astype(np.float64)
    t = transform_b[:3, 3].astype(np.float64)
    gts = src_b.astype(np.float64) @ R.T + t            # [N, 3]
    rec = recon_b.astype(np.float64)                    # [M, 3]

    xx = np.sum(gts * gts, axis=1)                      # [N]
    yy = np.sum(rec * rec, axis=1)                      # [M]

    # bf16 hi/lo pair decomposition of the cross-term factors
    g2 = (-2.0 * gts).astype(np.float32)                # [N, 3]
    g_hi = _bf16(g2)
    g_lo = _bf16(g2 - g_hi.astype(np.float32))
    p32 = rec.astype(np.float32)
    p_hi = _bf16(p32)
    p_lo = _bf16(p32 - p_hi.astype(np.float32))

    lhs = np.empty((11, NPTS), ml_dtypes.bfloat16)
    rhs = np.empty((11, NPTS), ml_dtypes.bfloat16)
    lhs[0:3] = g_hi.T
    lhs[3:6] = g_hi.T
    lhs[6:9] = g_lo.T
    lhs[9] = ml_dtypes.bfloat16(1.0)
    lhs[10] = _bf16(xx.astype(np.float32))
    rhs[0:3] = p_hi.T
    rhs[3:6] = p_lo.T
    rhs[6:9] = p_hi.T
    rhs[9] = _bf16(yy.astype(np.float32))
    rhs[10] = ml_dtypes.bfloat16(1.0)

    return {
        "lhsd": np.ascontiguousarray(lhs),
        "rhsd": np.ascontiguousarray(rhs),
        "ident": np.eye(128).astype(ml_dtypes.bfloat16),
    }


def kernel(recon, src_points, transform):
    global LAST_RESULTS
    recon = np.asarray(recon, np.float32)
    src_points = np.asarray(src_points, np.float32)
    transform = np.asarray(transform, np.float32)
    B = recon.shape[0]
    assert B == N_CORES

    if "nc" not in _CACHE:
        _CACHE["nc"] = _build_kernel()
    nc = _CACHE["nc"]

    in_maps = [
        _prep_core_inputs(recon[b], src_points[b], transform[b])
        for b in range(B)
    ]
    res = run_bass_kernel_spmd(nc, in_maps, list(range(N_CORES)))
    LAST_RESULTS = res
    total = np.float64(0.0)
    for r in res.results:
        total += np.float64(np.sum(r["partial"].astype(np.float64)))
    return np.float32(total)


# revision 56
# speedup vs baseline: 1.6816x; 1.0032x over previous
"""Chamfer loss kernel for Trainium2, batch-parallel over 8 NeuronCores.

Per core (one batch element b):
  gts = src_points[b] @ R^T + t          (host, fp64)
  P[i,j] = |gts_i|^2 + |recon_j|^2 - 2 gts_i . recon_j
  loss_b = sum_j min_i P + sum_i min_j P
Host sums the 8 partial losses.

Structure (v4):
- The host assembles the two K=11 bf16 matmul operands directly:
  rows 0-8 are the -2*g.p cross terms in a bf16 hi/lo pair decomposition
  (hi*hi + hi*lo + lo*hi, ~2^-18 relative accuracy), row 9 carries the
  yy_j norm (ones x yy), row 10 carries the xx_i norm (xx x ones). The
  norms are single bf16; their error is row/column-structured and cancels
  to ~1e-4 in the summed loss. The device sees ready operands, so the
  whole on-device prep phase is two ~90KB DMA loads.
- ONE K=11 bf16 matmul per 512-col chunk produces the COMPLETE distance
  tile in PSUM.
- PSUM exit: nearly all half-blocks leave PSUM through the ACT engine
  (copy -> bf16 SBUF) whose 1 elem/cyc/lane stream is the saturated
  bottleneck; per-row mins ride along on DVE tensor_scalar ops with a
  min accum_out (4x mode on the staged bf16, 0.25 cyc/elem), and the
  running per-column min is a 2x bf16 tensor_tensor per block, lagged
  one block so it never waits on the current block's exits. The last
  block's h0 exits through DVE tensor_scalar (stage + row-min in one
  1x op) so the final dependency chain is short.
- Per-column mins are finished with PE transposes + free-axis folds,
  pipelined against the last block's quarter-merges; the final
  partition sums ([128,2]) are added on the host.
"""

import os

# the axon client here has no NTFF profile hook; a stray BASS_TRACE=1 in the
# environment would crash run_bass_kernel_spmd on a missing import
os.environ["BASS_NEVER_TRACE"] = "1"

import ml_dtypes
import numpy as np

import concourse.bacc as bacc
import concourse.bass as bass
import concourse.mybir as mybir
import concourse.tile as tile
from concourse.bass_utils import run_bass_kernel_spmd

F32 = mybir.dt.float32
BF16 = mybir.dt.bfloat16
ALU = mybir.AluOpType
AX = mybir.AxisListType

N_CORES = 8
NPTS = 4096          # points per set (both gts and recon)
NBLK = NPTS // 128   # 32 row blocks
HALF = 2048          # P tile free width (4 PSUM banks)

# (block, half) units force-staged by DVE tensor_scalar (stage + row-min
# fused, 1x from PSUM). Empty measures best: the PSUM exits are emitted
# engine-unassigned and the tile scheduler splits them between ACT and
# DVE better than any hand-picked fusion set.
DVE_EXIT_HALVES = frozenset()

_CACHE = {}
LAST_RESULTS = None


def _build_kernel():
    nc = bacc.Bacc("TRN2", target_bir_lowering=False, debug=False)

    lhsd = nc.declare_dram_parameter("lhsd", [11, NPTS], BF16, isOutput=False)
    rhsd = nc.declare_dram_parameter("rhsd", [11, NPTS], BF16, isOutput=False)
    ident = nc.declare_dram_parameter("ident", [128, 128], BF16, isOutput=False)
    partial = nc.declare_dram_parameter("partial", [128, 2], F32, isOutput=True)

    with tile.TileContext(nc) as tc:
        with tc.tile_pool(name="sb", bufs=1) as sb:
            # ---- phase 0: load operands (two parallel DMA queues) -------
            # rhs rides the SP queue, lhs the ACT queue (idle this early);
            # the first distance matmuls need lhs cols 0:128 + rhs 0:2048
            lhs = sb.tile([11, NPTS], BF16)
            rhs = sb.tile([11, NPTS], BF16)
            ident_sb = sb.tile([128, 128], BF16)
            nc.sync.dma_start(out=ident_sb[:, :], in_=ident[:, :])
            nc.sync.dma_start(out=rhs[:, 0:2048], in_=rhsd[:, 0:2048])
            nc.scalar.dma_start(out=lhs[:, 0:2048], in_=lhsd[:, 0:2048])
            nc.sync.dma_start(out=rhs[:, 2048:4096], in_=rhsd[:, 2048:4096])
            nc.scalar.dma_start(out=lhs[:, 2048:4096], in_=lhsd[:, 2048:4096])

            # running reduction state (mrun needs no seed: block 0 copies
            # into it; rminA/B columns are fully written per block)
            mrun = sb.tile([128, NPTS], BF16)    # running col-min
            rminA = sb.tile([128, NBLK], F32)    # per-block h0 row mins
            rminB = sb.tile([128, NBLK], F32)    # per-block h1 row mins

            # PE warm-up on the identity while operands load, so the main
            # matmul stream starts at full PE clock
            with tc.tile_pool(name="warm_ps", bufs=1, space="PSUM") as wpp:
                warm_ps = wpp.tile([128, 128], F32)
                for _ in range(24):
                    nc.tensor.matmul(warm_ps[:, :], lhsT=ident_sb[:, :],
                                     rhs=ident_sb[:, :], start=True,
                                     stop=True)

            # ---- phase 1: distance tiles + fused min reductions ---------
            junk = sb.tile([128, NPTS], BF16)   # throwaway TS main output

            sg_last = sb.tile([128, NPTS], BF16)   # block 31 outlives pools

            with tc.tile_pool(name="stage_sb", bufs=8) as stg, \
                 tc.tile_pool(name="main_ps", bufs=2, space="PSUM") as mps:
                prev_sg = None
                for b in range(NBLK):
                    if b == NBLK - 1:
                        sg = sg_last
                        # block 30 is fully staged by now; merge it before
                        # block 31's DVE exit so the tail chain is shorter
                        nc.any.tensor_tensor(mrun[:, :], prev_sg[:, :],
                                             mrun[:, :], ALU.min)
                    else:
                        sg = stg.tile([128, NPTS], BF16, tag="SG")
                    lw = lhs[:, b * 128:(b + 1) * 128]
                    for h in range(2):
                        pt = mps.tile([128, HALF], F32, tag="P")
                        for s in range(HALF // 512):
                            j0 = h * HALF + s * 512
                            nc.tensor.matmul(
                                pt[:, s * 512:(s + 1) * 512], lhsT=lw,
                                rhs=rhs[:, j0:j0 + 512],
                                start=True, stop=True)
                        hs = slice(h * HALF, (h + 1) * HALF)
                        rm = rminA if h == 0 else rminB
                        if (b, h) in DVE_EXIT_HALVES:
                            # stage to bf16 + per-row min in one DVE op
                            nc.vector.tensor_scalar(
                                sg[:, hs], pt[:, :], 0.0, None, ALU.add,
                                ALU.min, accum_out=rm[:, b:b + 1])
                        elif b >= 26:
                            # near the tail, keep DVE free for its merge
                            # + fold chain: pin these exits to ACT
                            nc.scalar.copy(sg[:, hs], pt[:, :])
                            nc.any.tensor_scalar(
                                junk[:, hs], sg[:, hs], 0.0, None, ALU.add,
                                ALU.min, accum_out=rm[:, b:b + 1])
                        else:
                            nc.any.tensor_copy(sg[:, hs], pt[:, :])
                            # half-row min at 4x DVE rate, overlapping the
                            # other half's ACT exit; main output discarded
                            nc.any.tensor_scalar(
                                junk[:, hs], sg[:, hs], 0.0, None, ALU.add,
                                ALU.min, accum_out=rm[:, b:b + 1])
                        # col-min merge, lagged one block: while ACT exits
                        # block b, DVE merges block b-1 (no dependency on
                        # the current block, so DVE never waits on ACT)
                        if h == 0 and prev_sg is not None and b < NBLK - 1:
                            if b == 1:
                                nc.vector.tensor_copy(mrun[:, :],
                                                      prev_sg[:, :])
                            else:
                                nc.any.tensor_tensor(mrun[:, :],
                                                     prev_sg[:, :],
                                                     mrun[:, :], ALU.min)
                    prev_sg = sg

            # ---- phase 2: finishers -------------------------------------
            # per-partition sums go to the host, which adds the 256 floats
            psums = sb.tile([128, 2], F32)
            cmin = sb.tile([128, NPTS // 128], F32)

            with tc.tile_pool(name="fin_ps", bufs=4, space="PSUM") as fps:
                # block 31 merges in column quarters; each quarter's
                # transposes + fold start while the next quarter merges
                tps = []
                for g in range(4):
                    qs = slice(g * 1024, (g + 1) * 1024)
                    nc.vector.tensor_tensor(mrun[:, qs], sg_last[:, qs],
                                            mrun[:, qs], ALU.min)
                    tp = fps.tile([128, 1024], BF16, tag="T")
                    for c in range(8):
                        j0 = (g * 8 + c) * 128
                        nc.tensor.transpose(tp[:, c * 128:(c + 1) * 128],
                                            mrun[:, j0:j0 + 128],
                                            ident_sb[:, :])
                    tps.append(tp)
                # row-side combine slots into the transpose wait
                nc.vector.tensor_tensor(rminA[:, :], rminA[:, :],
                                        rminB[:, :], ALU.min)
                nc.vector.tensor_reduce(psums[:, 0:1], rminA[:, :],
                                        axis=AX.X, op=ALU.add)
                nc.sync.dma_start(out=partial[:, 0:1], in_=psums[:, 0:1])
                for g in range(4):
                    nc.vector.tensor_reduce(
                        cmin[:, 8 * g:8 * g + 8],
                        tps[g].rearrange("p (g w) -> p g w", w=128),
                        axis=AX.X, op=ALU.min)
                nc.vector.tensor_reduce(psums[:, 1:2], cmin[:, :], axis=AX.X,
                                        op=ALU.add)

            nc.sync.dma_start(out=partial[:, 1:2], in_=psums[:, 1:2])

    nc.compile()
    return nc


def _bf16(x):
    return x.astype(ml_dtypes.bfloat16)


def _prep_core_inputs(recon_b, src_b, transform_b):
    # transform on host at fp64: gts = src @ R^T + t
    R = transform_b[:3, :3].astype(np.float64)
    t = transform_b[:3, 3].astype(np.float64)
    gts = src_b.astype(np.float64) @ R.T + t            # [N, 3]
    rec = recon_b.astype(np.float64)                    # [M, 3]

    xx = np.sum(gts * gts, axis=1)                      # [N]
    yy = np.sum(rec * rec, axis=1)                      # [M]

    # bf16 hi/lo pair decomposition of the cross-term factors
    g2 = (-2.0 * gts).astype(np.float32)                # [N, 3]
    g_hi = _bf16(g2)
    g_lo = _bf16(g2 - g_hi.astype(np.float32))
    p32 = rec.astype(np.float32)
    p_hi = _bf16(p32)
    p_lo = _bf16(p32 - p_hi.astype(np.float32))

    lhs = np.empty((11, NPTS), ml_dtypes.bfloat16)
    rhs = np.empty((11, NPTS), ml_dtypes.bfloat16)
    lhs[0:3] = g_hi.T
    lhs[3:6] = g_hi.T
    lhs[6:9] = g_lo.T
    lhs[9] = ml_dtypes.bfloat16(1.0)
    lhs[10] = _bf16(xx.astype(np.float32))
    rhs[0:3] = p_hi.T
    rhs[3:6] = p_lo.T
    rhs[6:9] = p_hi.T
    rhs[9] = _bf16(yy.astype(np.float32))
    rhs[10] = ml_dtypes.bfloat16(1.0)

    return {
        "lhsd": np.ascontiguousarray(lhs),
        "rhsd": np.ascontiguousarray(rhs),
        "ident": np.eye(128).astype(ml_dtypes.bfloat16),
    }


def kernel(recon, src_points, transform):
    global LAST_RESULTS
    recon = np.asarray(recon, np.float32)
    src_points = np.asarray(src_points, np.float32)
    transform = np.asarray(transform, np.float32)
    B = recon.shape[0]
    assert B == N_CORES

    if "nc" not in _CACHE:
        _CACHE["nc"] = _build_kernel()
    nc = _CACHE["nc"]

    in_maps = [
        _prep_core_inputs(recon[b], src_points[b], transform[b])
        for b in range(B)
    ]
    res = run_bass_kernel_spmd(nc, in_maps, list(range(N_CORES)))
    LAST_RESULTS = res
    total = np.float64(0.0)
    for r in res.results:
        total += np.float64(np.sum(r["partial"].astype(np.float64)))
    return np.float32(total)
